# revision 8
# baseline (speedup 1.0000x reference)
"""Trainium2 Bass kernel for ChannelSqueezeSpatialAttention.

Reference computation (shapes hardcoded):
  xq  [4, 256, 64, 64], xkv [4, 256, 32, 32]
  wq/wk/wv [256, 256], emb_q/emb_k [17, 64, 3, 7, 7]
  q = wq @ xq (1x1 conv), k = wk @ xkv, v = wv @ xkv
  q_c = conv3d(q, emb_q) over (head, y, x) with kernel (3,7,7) -> 17 ch/head
  k_c = conv3d(k, emb_k)
  sim = softmax(q_c^T k_c / 8), rec = sim @ v  -> [4, 256, 64, 64]

Sharding: 8 cores = 4 batches x 2 head-pairs. Each core computes 2 heads of
one batch. The conv mixes adjacent heads (3-wide along head axis), so each
core computes q/k projections for its pair-relative head slots r0..r3 =
heads (2p-1, 2p, 2p+1, 2p+2); out-of-range slots get zero weight columns
host-side (no halo exchange needed).

Conv mapping: shift-and-accumulate matmuls with M = (ky, cg) = 7*17 = 119
packed output rows. The ky-summation is deferred: partial planes are stored
to SBUF with a per-ky y-shift (small SBUF->SBUF DMAs), and the scores
matmul contracts over (ky, cg) with a ky-replicated k_c as the stationary
operand, which completes the convolution for free.

Attention: scores computed transposed S^T[sk, sq] so softmax-exp output E^T
feeds the value matmul directly: rec^T[d|Z, sq] = [v|1]^T E^T. Division by
Z via DVE reciprocal + K=1 broadcast matmul + DVE multiply.

All matmuls run in float32r (fp32 streamed at bf16 rate, ~1e-4 rel err).
"""

import functools
import numpy as np

import concourse.bass as bass
import concourse.tile as tile
import concourse.mybir as mybir
from concourse import bacc
from concourse.bass_utils import run_bass_kernel_spmd

F32 = mybir.dt.float32
F32R = mybir.dt.float32r

B = 4
NH = 4
D = 64            # head dim
CG = 17           # squeezed channels
K7 = 7            # spatial kernel
HQ = 64           # q image h=w
HK = 32           # k image h=w
SQ = HQ * HQ      # 4096
SK = HK * HK      # 1024
MC = K7 * CG      # 119 conv output rows (ky, cg)
QP = HQ + 6       # 70: x-padded q row width
KP = HK + 6       # 38: x-padded k row width
SCALE = D ** -0.5

QCH = 8           # q spatial chunks (8 y-rows each)
KCH = 2           # k spatial chunks (16 y-rows each)
QROWS = HQ // QCH  # 8
KROWS = HK // KCH  # 16
NSLAB = SQ // 512  # 8 sq slabs per head


def _build_program():
    nc = bacc.Bacc()

    xq = nc.dram_tensor("xq", [256, SQ], F32R, kind="ExternalInput")
    xkv = nc.dram_tensor("xkv", [256, SK], F32R, kind="ExternalInput")
    wqT = nc.dram_tensor("wqT", [256, 256], F32R, kind="ExternalInput")
    wkT = nc.dram_tensor("wkT", [256, 256], F32R, kind="ExternalInput")
    wvT = nc.dram_tensor("wvT", [256, 128], F32R, kind="ExternalInput")
    wcq = nc.dram_tensor("wcq", [128, 7, MC], F32R, kind="ExternalInput")
    wcq2 = nc.dram_tensor("wcq2", [128, 7, MC], F32R, kind="ExternalInput")
    wck = nc.dram_tensor("wck", [128, 7, MC], F32R, kind="ExternalInput")
    wck2 = nc.dram_tensor("wck2", [128, 7, MC], F32R, kind="ExternalInput")
    repl = nc.dram_tensor("repl", [MC, MC], F32R, kind="ExternalInput")
    out = nc.dram_tensor("out", [128, SQ], F32, kind="ExternalOutput")

    with tile.TileContext(nc) as tc:
        _emit(nc, tc, xq, xkv, wqT, wkT, wvT, wcq, wcq2, wck, wck2, repl, out)
    nc.compile()
    return nc


def _emit(nc, tc, xq, xkv, wqT, wkT, wvT, wcq, wcq2, wck, wck2, repl, out):
    import contextlib
    ctx = contextlib.ExitStack()
    with ctx:
        consts = ctx.enter_context(tc.tile_pool(name="consts", bufs=1))
        xqp = ctx.enter_context(tc.tile_pool(name="xqp", bufs=3))
        stg = ctx.enter_context(tc.tile_pool(name="stg", bufs=3))
        pqp = ctx.enter_context(tc.tile_pool(name="pqp", bufs=1))
        pkp = ctx.enter_context(tc.tile_pool(name="pkp", bufs=1))
        k2p = ctx.enter_context(tc.tile_pool(name="k2p", bufs=1))
        ep = ctx.enter_context(tc.tile_pool(name="ep", bufs=2))
        zp = ctx.enter_context(tc.tile_pool(name="zp", bufs=2))
        op = ctx.enter_context(tc.tile_pool(name="op", bufs=3))
        ps_mm = ctx.enter_context(tc.tile_pool(name="ps_mm", bufs=2, space="PSUM"))
        ps_sc = ctx.enter_context(tc.tile_pool(name="ps_sc", bufs=1, space="PSUM"))
        ps_rec = ctx.enter_context(tc.tile_pool(name="ps_rec", bufs=2, space="PSUM"))

        # ---- constant loads ----
        wqT_sb = consts.tile([128, 2, 256], F32R)
        wkT_sb = consts.tile([128, 2, 256], F32R)
        wvT_sb = consts.tile([128, 2, 128], F32R)
        nc.sync.dma_start(wqT_sb, wqT.rearrange("(t p) m -> p t m", t=2))
        nc.sync.dma_start(wkT_sb, wkT.rearrange("(t p) m -> p t m", t=2))
        nc.sync.dma_start(wvT_sb, wvT.rearrange("(t p) m -> p t m", t=2))
        wcq_sb = consts.tile([128, 7, MC], F32R)
        wcq2_sb = consts.tile([128, 7, MC], F32R)
        wck_sb = consts.tile([128, 7, MC], F32R)
        wck2_sb = consts.tile([128, 7, MC], F32R)
        nc.sync.dma_start(wcq_sb, wcq[:])
        nc.sync.dma_start(wcq2_sb, wcq2[:])
        nc.sync.dma_start(wck_sb, wck[:])
        nc.sync.dma_start(wck2_sb, wck2[:])
        repl_sb = consts.tile([MC, MC], F32R)
        nc.sync.dma_start(repl_sb, repl[:])
        # ones row at partition 64 (aligned with Z row of rec psum)
        ones_t = consts.tile([65, 64], F32R)
        nc.vector.memset(ones_t[64:65, :].bitcast(F32), 1.0)
        xkv_sb = consts.tile([128, 2, SK], F32R)
        nc.sync.dma_start(xkv_sb, xkv.rearrange("(t p) n -> p t n", t=2))

        # q/k plane tensors: [r0,r1], [r1,r2], [r2,r3]; x-padded rows (64|32) x (70|38)
        qA = consts.tile([128, HQ * QP], F32R)
        qB = consts.tile([128, HQ * QP], F32R)
        qC = consts.tile([128, HQ * QP], F32R)
        kA = consts.tile([128, HK * KP], F32R)
        kB = consts.tile([128, HK * KP], F32R)
        kC = consts.tile([128, HK * KP], F32R)

        # x-border zeros (cols 0..2 and 67..69 of each padded row); B copies
        # inherit them from A/C.
        for t in (qA, qC):
            v = t[:].rearrange("p (r c) -> p r c", c=QP)
            nc.vector.memset(v[:, :, 0:3].bitcast(F32), 0.0)
            nc.vector.memset(v[:, :, QP - 3:QP].bitcast(F32), 0.0)
        for t in (kA, kC):
            v = t[:].rearrange("p (r c) -> p r c", c=KP)
            nc.vector.memset(v[:, :, 0:3].bitcast(F32), 0.0)
            nc.vector.memset(v[:, :, KP - 3:KP].bitcast(F32), 0.0)

        # v^T projection: out[sk_blk, (h0 d | h1 d)] ; lhsT = xkv [c, sk], rhs = wvT [c, d2]
        v_sb = consts.tile([128, 8, 130], F32R)
        nc.vector.memset(v_sb[:, :, 64:65].bitcast(F32), 1.0)
        nc.vector.memset(v_sb[:, :, 129:130].bitcast(F32), 1.0)
        for t in range(8):
            acc = ps_mm.tile([128, 512], F32, tag="mm")
            for ct in range(2):
                nc.tensor.matmul(
                    acc[:, 0:128],
                    xkv_sb[:, ct, t * 128:(t + 1) * 128],
                    wvT_sb[:, ct, :],
                    start=(ct == 0), stop=(ct == 1),
                )
            nc.vector.tensor_copy(v_sb[:, t, 0:64], acc[:, 0:64])
            nc.vector.tensor_copy(v_sb[:, t, 65:129], acc[:, 64:128])

        # k projection -> kA/kC padded planes
        for ch in range(2):
            for mt in range(2):
                acc = ps_mm.tile([128, 512], F32, tag="mm")
                for ct in range(2):
                    nc.tensor.matmul(
                        acc[:],
                        wkT_sb[:, ct, mt * 128:(mt + 1) * 128],
                        xkv_sb[:, ct, ch * 512:(ch + 1) * 512],
                        start=(ct == 0), stop=(ct == 1),
                    )
                rows = 512 // HK  # 16
                y0 = ch * rows
                full = (kA, kC)[mt]
                dstf = full[:].rearrange("p (r c) -> p r c", c=KP)
                accv = acc[:].rearrange("p (r c) -> p r c", c=HK)
                nc.vector.tensor_copy(dstf[:, y0:y0 + rows, 3:3 + HK], accv)

        # q projection -> qA/qC
        for ch in range(QCH):
            xt = [None, None]
            for ct in range(2):
                xt[ct] = xqp.tile([128, 512], F32R, tag="xq", name="xqt")
                nc.sync.dma_start(xt[ct], xq[ct * 128:(ct + 1) * 128,
                                              ch * 512:(ch + 1) * 512])
            for mt in range(2):
                acc = ps_mm.tile([128, 512], F32, tag="mm")
                for ct in range(2):
                    nc.tensor.matmul(
                        acc[:],
                        wqT_sb[:, ct, mt * 128:(mt + 1) * 128],
                        xt[ct][:],
                        start=(ct == 0), stop=(ct == 1),
                    )
                rows = 512 // HQ  # 8
                y0 = ch * rows
                full = (qA, qC)[mt]
                dstf = full[:].rearrange("p (r c) -> p r c", c=QP)
                accv = acc[:].rearrange("p (r c) -> p r c", c=HQ)
                nc.vector.tensor_copy(dstf[:, y0:y0 + rows, 3:3 + HQ], accv)

        # B planes (r1, r2) via partition-shifting SBUF->SBUF DMA
        nc.sync.dma_start(qB[0:64, :], qA[64:128, :])
        nc.sync.dma_start(qB[64:128, :], qC[0:64, :])
        nc.sync.dma_start(kB[0:64, :], kA[64:128, :])
        nc.sync.dma_start(kB[64:128, :], kC[0:64, :])

        # ---- per-head conv + attention ----
        for h in range(2):
            q128 = (qA, qB)[h]          # K=128 plane pair for this head
            k128 = (kA, kB)[h]
            sl = slice(0, 64) if h == 0 else slice(64, 128)  # third plane in C

            # conv_k partials -> shifted Pk
            pk = pkp.tile([MC, SK], F32R, tag="pk")
            nc.vector.memset(pk[:, 0:3 * HK].bitcast(F32), 0.0)
            nc.vector.memset(pk[:, (HK - 3) * HK:SK].bitcast(F32), 0.0)
            for ch in range(KCH):
                acc = ps_mm.tile([MC, 512], F32, tag="mm")
                y0 = ch * KROWS
                for kx in range(K7):
                    r1 = k128[:].rearrange("p (r c) -> p r c", c=KP)[
                        :, y0:y0 + KROWS, kx:kx + HK]
                    nc.tensor.matmul(acc[:], wck_sb[:, kx, :], r1,
                                     start=(kx == 0), stop=False)
                for kx in range(K7):
                    r2 = kC[:].rearrange("p (r c) -> p r c", c=KP)[
                        sl, y0:y0 + KROWS, kx:kx + HK]
                    nc.tensor.matmul(acc[:], wck2_sb[sl, kx, :], r2,
                                     start=False, stop=(kx == K7 - 1))
                st = stg.tile([MC, 512], F32R, tag="stg")
                nc.vector.tensor_copy(st[:], acc[:])
                # shifted stores: group g rows yy -> dest y = yy - g + 3
                for g in range(K7):
                    i0 = max(0, g - 3 - y0)
                    i1 = min(KROWS, HK + g - 3 - y0)
                    if i1 <= i0:
                        continue
                    cnt = (i1 - i0) * HK
                    d0 = (y0 + i0 - g + 3) * HK
                    nc.sync.dma_start(
                        pk[g * CG:(g + 1) * CG, d0:d0 + cnt],
                        st[g * CG:(g + 1) * CG, i0 * HK:i0 * HK + cnt],
                    )

            # K2 = ky-replicated k_c
            k2 = k2p.tile([MC, SK], F32R, tag="k2")
            for ch in range(2):
                acc = ps_mm.tile([MC, 512], F32, tag="mm")
                nc.tensor.matmul(acc[:], repl_sb[:],
                                 pk[:, ch * 512:(ch + 1) * 512],
                                 start=True, stop=True)
                nc.vector.tensor_copy(k2[:, ch * 512:(ch + 1) * 512], acc[:])

            # conv_q partials -> shifted Pq
            pq = pqp.tile([MC, SQ], F32R, tag="pq")
            nc.vector.memset(pq[:, 0:3 * HQ].bitcast(F32), 0.0)
            nc.vector.memset(pq[:, (HQ - 3) * HQ:SQ].bitcast(F32), 0.0)
            for ch in range(QCH):
                acc = ps_mm.tile([MC, 512], F32, tag="mm")
                y0 = ch * QROWS
                for kx in range(K7):
                    r1 = q128[:].rearrange("p (r c) -> p r c", c=QP)[
                        :, y0:y0 + QROWS, kx:kx + HQ]
                    nc.tensor.matmul(acc[:], wcq_sb[:, kx, :], r1,
                                     start=(kx == 0), stop=False)
                for kx in range(K7):
                    r2 = qC[:].rearrange("p (r c) -> p r c", c=QP)[
                        sl, y0:y0 + QROWS, kx:kx + HQ]
                    nc.tensor.matmul(acc[:], wcq2_sb[sl, kx, :], r2,
                                     start=False, stop=(kx == K7 - 1))
                st = stg.tile([MC, 512], F32R, tag="stg")
                nc.vector.tensor_copy(st[:], acc[:])
                for g in range(K7):
                    i0 = max(0, g - 3 - y0)
                    i1 = min(QROWS, HQ + g - 3 - y0)
                    if i1 <= i0:
                        continue
                    cnt = (i1 - i0) * HQ
                    d0 = (y0 + i0 - g + 3) * HQ
                    nc.sync.dma_start(
                        pq[g * CG:(g + 1) * CG, d0:d0 + cnt],
                        st[g * CG:(g + 1) * CG, i0 * HQ:i0 * HQ + cnt],
                    )

            # attention slabs
            for s in range(NSLAB):
                e_sb = ep.tile([128, SQ], F32R, tag="e")
                for half in range(2):
                    sc = ps_sc.tile([128, 2048], F32, tag="sc")
                    for bb in range(4):
                        blk = half * 4 + bb
                        nc.tensor.matmul(
                            sc[:, bb * 512:(bb + 1) * 512],
                            k2[:, blk * 128:(blk + 1) * 128],
                            pq[:, s * 512:(s + 1) * 512],
                            start=True, stop=True,
                        )
                    nc.scalar.activation(
                        e_sb[:, half * 2048:(half + 1) * 2048], sc[:],
                        mybir.ActivationFunctionType.Exp, scale=SCALE)
                rec = ps_rec.tile([65, 512], F32, tag="rec")
                for t in range(8):
                    nc.tensor.matmul(
                        rec[:],
                        v_sb[:, t, h * 65:(h + 1) * 65],
                        e_sb[:, t * 512:(t + 1) * 512],
                        start=(t == 0), stop=(t == 7),
                    )
                zr = zp.tile([65, 512], F32R, tag="zr")
                with nc.allow_low_precision("fp32r z-reciprocal"):
                    nc.vector.reciprocal(zr[64:65, :], rec[64:65, :])
                zb = ps_mm.tile([64, 512], F32, tag="mm")
                nc.tensor.matmul(zb[:], ones_t[64:65, :], zr[64:65, :],
                                 start=True, stop=True)
                zb_sb = zp.tile([64, 512], F32, tag="zbs")
                nc.vector.tensor_copy(zb_sb[:], zb[:])
                ot = op.tile([64, 512], F32, tag="ot")
                nc.vector.tensor_mul(ot[:], rec[0:64, :], zb_sb[:])
                nc.sync.dma_start(
                    out[h * 64:(h + 1) * 64, s * 512:(s + 1) * 512], ot[:])


@functools.lru_cache(maxsize=1)
def _get_program():
    return _build_program()


def _host_inputs(xq, xkv, wq, wk, wv, emb_q, emb_k):
    """Build the 8 per-core input maps."""
    xq = np.ascontiguousarray(xq, dtype=np.float32)
    xkv = np.ascontiguousarray(xkv, dtype=np.float32)

    def conv_w(emb):
        # emb [cg, d, dnk, ky, kx] -> rows (dnk, d), cols (kx, ky, cg)
        arr = np.transpose(np.asarray(emb, np.float32), (2, 1, 4, 3, 0))
        w128 = np.ascontiguousarray(arr[0:2].reshape(128, 7, MC))
        w64 = arr[2].reshape(64, 7, MC)
        w64d = np.ascontiguousarray(
            np.concatenate([w64, w64], axis=0))  # duplicated halves
        return w128, w64d

    wcq_, wcq2_ = conv_w(emb_q)
    wck_, wck2_ = conv_w(emb_k)
    repl_ = np.ascontiguousarray(
        np.tile(np.eye(CG, dtype=np.float32), (K7, K7)))

    wq = np.asarray(wq, np.float32)
    wk = np.asarray(wk, np.float32)
    wv = np.asarray(wv, np.float32)

    in_maps = []
    for core in range(8):
        b, p = divmod(core, 2)
        wqT_ = np.zeros((256, 256), np.float32)
        wkT_ = np.zeros((256, 256), np.float32)
        for j in range(4):
            head = 2 * p + j - 1
            if 0 <= head < NH:
                wqT_[:, j * 64:(j + 1) * 64] = wq[head * 64:(head + 1) * 64, :].T
                wkT_[:, j * 64:(j + 1) * 64] = wk[head * 64:(head + 1) * 64, :].T
        wvT_ = np.ascontiguousarray(wv[p * 128:(p + 1) * 128, :].T)
        in_maps.append(dict(
            xq=np.ascontiguousarray(xq[b].reshape(256, SQ)),
            xkv=np.ascontiguousarray(xkv[b].reshape(256, SK)),
            wqT=wqT_, wkT=wkT_, wvT=wvT_,
            wcq=wcq_, wcq2=wcq2_, wck=wck_, wck2=wck2_,
            repl=repl_,
        ))
    return in_maps


def _run(inputs, **kw):
    nc = _get_program()
    in_maps = _host_inputs(**inputs)
    res = run_bass_kernel_spmd(nc, in_maps, core_ids=list(range(8)), **kw)
    outp = np.empty((B, 256, HQ, HQ), np.float32)
    for core in range(8):
        b, p = divmod(core, 2)
        outp[b, p * 128:(p + 1) * 128] = \
            res.results[core]["out"].reshape(128, HQ, HQ)
    return outp, res


def kernel(xq, xkv, wq, wk, wv, emb_q, emb_k):
    outp, _ = _run(dict(xq=xq, xkv=xkv, wq=wq, wk=wk, wv=wv,
                        emb_q=emb_q, emb_k=emb_k))
    return outp


# revision 9
# speedup vs baseline: 1.3252x; 1.3252x over previous
"""Trainium2 Bass kernel for ChannelSqueezeSpatialAttention.

Reference computation (shapes hardcoded):
  xq  [4, 256, 64, 64], xkv [4, 256, 32, 32]
  wq/wk/wv [256, 256], emb_q/emb_k [17, 64, 3, 7, 7]
  q = wq @ xq (1x1 conv), k = wk @ xkv, v = wv @ xkv
  q_c = conv3d(q, emb_q) over (head, y, x) with kernel (3,7,7) -> 17 ch/head
  k_c = conv3d(k, emb_k)
  sim = softmax(q_c^T k_c / 8), rec = sim @ v  -> [4, 256, 64, 64]

Sharding: 8 cores = 4 batches x 2 head-pairs. Each core computes 2 heads of
one batch. The conv mixes adjacent heads (3-wide along head axis), so each
core computes q/k projections for its pair-relative head slots r0..r3 =
heads (2p-1, 2p, 2p+1, 2p+2); out-of-range slots get zero weight columns
host-side (no halo exchange needed).

Conv mapping: shift-and-accumulate matmuls with M = (ky, cg) = 7*17 = 119
packed output rows. The ky-summation is deferred: partial planes are stored
to SBUF with a per-ky y-shift (small SBUF->SBUF DMAs), and the scores
matmul contracts over (ky, cg) with a ky-replicated k_c as the stationary
operand, which completes the convolution for free.

Attention: scores computed transposed S^T[sk, sq] so softmax-exp output E^T
feeds the value matmul directly: rec^T[d|Z, sq] = [v|1]^T E^T. Division by
Z via ACT ln/exp (1/Z = exp(-ln Z), same table set as the softmax exp) +
K=1 broadcast matmul + DVE multiply.

Dtypes: conv/scores chain in bf16 (PE streams 2-byte dtypes at 1 row/cycle
vs 2 for fp32r); projections, E, and the value matmul in float32r.
"""

import functools
import numpy as np
import ml_dtypes

import concourse.bass as bass
import concourse.tile as tile
import concourse.mybir as mybir
from concourse import bacc
from concourse.bass_utils import run_bass_kernel_spmd

F32 = mybir.dt.float32
F32R = mybir.dt.float32r
BF16 = mybir.dt.bfloat16

B = 4
NH = 4
D = 64            # head dim
CG = 17           # squeezed channels
K7 = 7            # spatial kernel
HQ = 64           # q image h=w
HK = 32           # k image h=w
SQ = HQ * HQ      # 4096
SK = HK * HK      # 1024
MC = K7 * CG      # 119 conv output rows (ky, cg)
QP = HQ + 6       # 70: x-padded q row width
KP = HK + 6       # 38: x-padded k row width
SCALE = D ** -0.5

QCH = 8           # q spatial chunks (8 y-rows each)
KCH = 2           # k spatial chunks (16 y-rows each)
QROWS = HQ // QCH  # 8
KROWS = HK // KCH  # 16
NSLAB = SQ // 512  # 8 sq slabs per head

AF = mybir.ActivationFunctionType


def _build_program():
    nc = bacc.Bacc()

    xq = nc.dram_tensor("xq", [256, SQ], F32R, kind="ExternalInput")
    xkv = nc.dram_tensor("xkv", [256, SK], F32R, kind="ExternalInput")
    wqT = nc.dram_tensor("wqT", [256, 256], F32R, kind="ExternalInput")
    wkT = nc.dram_tensor("wkT", [256, 256], F32R, kind="ExternalInput")
    wvT = nc.dram_tensor("wvT", [256, 128], F32R, kind="ExternalInput")
    wcq = nc.dram_tensor("wcq", [128, 7, MC], BF16, kind="ExternalInput")
    wcq2 = nc.dram_tensor("wcq2", [128, 7, MC], BF16, kind="ExternalInput")
    wck = nc.dram_tensor("wck", [128, 7, MC], BF16, kind="ExternalInput")
    wck2 = nc.dram_tensor("wck2", [128, 7, MC], BF16, kind="ExternalInput")
    repl = nc.dram_tensor("repl", [MC, MC], BF16, kind="ExternalInput")
    out = nc.dram_tensor("out", [128, SQ], F32, kind="ExternalOutput")

    with tile.TileContext(nc) as tc:
        _emit(nc, tc, xq, xkv, wqT, wkT, wvT, wcq, wcq2, wck, wck2, repl, out)
    nc.compile()
    return nc


def _emit(nc, tc, xq, xkv, wqT, wkT, wvT, wcq, wcq2, wck, wck2, repl, out):
    import contextlib
    ctx = contextlib.ExitStack()
    with ctx:
        consts = ctx.enter_context(tc.tile_pool(name="consts", bufs=1))
        xqp = ctx.enter_context(tc.tile_pool(name="xqp", bufs=3))
        stg = ctx.enter_context(tc.tile_pool(name="stg", bufs=3))
        pqp = ctx.enter_context(tc.tile_pool(name="pqp", bufs=2))
        pkp = ctx.enter_context(tc.tile_pool(name="pkp", bufs=2))
        k2p = ctx.enter_context(tc.tile_pool(name="k2p", bufs=2))
        ep = ctx.enter_context(tc.tile_pool(name="ep", bufs=2))
        rp = ctx.enter_context(tc.tile_pool(name="rp", bufs=2))
        zp = ctx.enter_context(tc.tile_pool(name="zp", bufs=1))
        op = ctx.enter_context(tc.tile_pool(name="op", bufs=3))
        ps_mm = ctx.enter_context(tc.tile_pool(name="ps_mm", bufs=2, space="PSUM"))
        ps_sc = ctx.enter_context(tc.tile_pool(name="ps_sc", bufs=1, space="PSUM"))
        ps_rec = ctx.enter_context(tc.tile_pool(name="ps_rec", bufs=2, space="PSUM"))

        # ---- constant loads ----
        wqT_sb = consts.tile([128, 2, 256], F32R)
        wkT_sb = consts.tile([128, 2, 256], F32R)
        wvT_sb = consts.tile([128, 2, 128], F32R)
        nc.sync.dma_start(wqT_sb, wqT.rearrange("(t p) m -> p t m", t=2))
        nc.sync.dma_start(wkT_sb, wkT.rearrange("(t p) m -> p t m", t=2))
        nc.sync.dma_start(wvT_sb, wvT.rearrange("(t p) m -> p t m", t=2))
        wcq_sb = consts.tile([128, 7, MC], BF16)
        wcq2_sb = consts.tile([128, 7, MC], BF16)
        wck_sb = consts.tile([128, 7, MC], BF16)
        wck2_sb = consts.tile([128, 7, MC], BF16)
        nc.sync.dma_start(wcq_sb, wcq[:])
        nc.sync.dma_start(wcq2_sb, wcq2[:])
        nc.sync.dma_start(wck_sb, wck[:])
        nc.sync.dma_start(wck2_sb, wck2[:])
        repl_sb = consts.tile([MC, MC], BF16)
        nc.sync.dma_start(repl_sb, repl[:])
        # ones row at partition 64 (aligned with Z row of rec psum)
        ones_t = consts.tile([65, 64], F32R)
        nc.vector.memset(ones_t[64:65, :].bitcast(F32), 1.0)
        xkv_sb = consts.tile([128, 2, SK], F32R)
        nc.sync.dma_start(xkv_sb, xkv.rearrange("(t p) n -> p t n", t=2))

        # q/k plane tensors: [r0,r1], [r1,r2], [r2,r3]; x-padded (64|32)x(70|38)
        qA = consts.tile([128, HQ * QP], BF16)
        qB = consts.tile([128, HQ * QP], BF16)
        qC = consts.tile([128, HQ * QP], BF16)
        kA = consts.tile([128, HK * KP], BF16)
        kB = consts.tile([128, HK * KP], BF16)
        kC = consts.tile([128, HK * KP], BF16)

        # x-border zeros (cols 0..2 and 67..69 of each padded row); B copies
        # inherit them from A/C.
        for t in (qA, qC):
            v = t[:].rearrange("p (r c) -> p r c", c=QP)
            nc.vector.memset(v[:, :, 0:3], 0.0)
            nc.vector.memset(v[:, :, QP - 3:QP], 0.0)
        for t in (kA, kC):
            v = t[:].rearrange("p (r c) -> p r c", c=KP)
            nc.vector.memset(v[:, :, 0:3], 0.0)
            nc.vector.memset(v[:, :, KP - 3:KP], 0.0)

        # v^T projection: out[sk_blk, (h0 d | h1 d)]
        v_sb = consts.tile([128, 8, 130], F32R)
        nc.vector.memset(v_sb[:, :, 64:65].bitcast(F32), 1.0)
        nc.vector.memset(v_sb[:, :, 129:130].bitcast(F32), 1.0)
        for t in range(8):
            acc = ps_mm.tile([128, 512], F32, tag="mm", name="accv")
            for ct in range(2):
                nc.tensor.matmul(
                    acc[:, 0:128],
                    xkv_sb[:, ct, t * 128:(t + 1) * 128],
                    wvT_sb[:, ct, :],
                    start=(ct == 0), stop=(ct == 1),
                )
            nc.vector.tensor_copy(v_sb[:, t, 0:64], acc[:, 0:64])
            nc.vector.tensor_copy(v_sb[:, t, 65:129], acc[:, 64:128])

        # k projection -> kA/kC padded planes (bf16)
        for ch in range(2):
            for mt in range(2):
                acc = ps_mm.tile([128, 512], F32, tag="mm", name="acckp")
                for ct in range(2):
                    nc.tensor.matmul(
                        acc[:],
                        wkT_sb[:, ct, mt * 128:(mt + 1) * 128],
                        xkv_sb[:, ct, ch * 512:(ch + 1) * 512],
                        start=(ct == 0), stop=(ct == 1),
                    )
                rows = 512 // HK  # 16
                y0 = ch * rows
                full = (kA, kC)[mt]
                dstf = full[:].rearrange("p (r c) -> p r c", c=KP)
                accv = acc[:].rearrange("p (r c) -> p r c", c=HK)
                nc.vector.tensor_copy(dstf[:, y0:y0 + rows, 3:3 + HK], accv)

        # q projection -> qA/qC (bf16)
        for ch in range(QCH):
            xt = [None, None]
            for ct in range(2):
                xt[ct] = xqp.tile([128, 512], F32R, tag="xq", name="xqt")
                nc.sync.dma_start(xt[ct], xq[ct * 128:(ct + 1) * 128,
                                              ch * 512:(ch + 1) * 512])
            for mt in range(2):
                acc = ps_mm.tile([128, 512], F32, tag="mm", name="accqp")
                for ct in range(2):
                    nc.tensor.matmul(
                        acc[:],
                        wqT_sb[:, ct, mt * 128:(mt + 1) * 128],
                        xt[ct][:],
                        start=(ct == 0), stop=(ct == 1),
                    )
                rows = 512 // HQ  # 8
                y0 = ch * rows
                full = (qA, qC)[mt]
                dstf = full[:].rearrange("p (r c) -> p r c", c=QP)
                accv = acc[:].rearrange("p (r c) -> p r c", c=HQ)
                nc.vector.tensor_copy(dstf[:, y0:y0 + rows, 3:3 + HQ], accv)

        # B planes (r1, r2) via partition-shifting SBUF->SBUF DMA
        nc.sync.dma_start(qB[0:64, :], qA[64:128, :])
        nc.sync.dma_start(qB[64:128, :], qC[0:64, :])
        nc.sync.dma_start(kB[0:64, :], kA[64:128, :])
        nc.sync.dma_start(kB[64:128, :], kC[0:64, :])

        # ---- per-head stage emitters ----
        pq_t = [None, None]
        pk_t = [None, None]
        k2_t = [None, None]
        rec_sb_t = [None, None]

        def conv_k_chunk(h, ch):
            k128 = (kA, kB)[h]
            sl = slice(0, 64) if h == 0 else slice(64, 128)
            if ch == 0:
                pk = pkp.tile([MC, SK], BF16, tag="pk", name="pk")
                nc.vector.memset(pk[:, 0:3 * HK], 0.0)
                nc.vector.memset(pk[:, (HK - 3) * HK:SK], 0.0)
                pk_t[h] = pk
            pk = pk_t[h]
            acc = ps_mm.tile([MC, 512], F32, tag="mm", name="acck")
            y0 = ch * KROWS
            for kx in range(K7):
                r1 = k128[:].rearrange("p (r c) -> p r c", c=KP)[
                    :, y0:y0 + KROWS, kx:kx + HK]
                nc.tensor.matmul(acc[:], wck_sb[:, kx, :], r1,
                                 start=(kx == 0), stop=False)
            for kx in range(K7):
                r2 = kC[:].rearrange("p (r c) -> p r c", c=KP)[
                    sl, y0:y0 + KROWS, kx:kx + HK]
                nc.tensor.matmul(acc[:], wck2_sb[sl, kx, :], r2,
                                 start=False, stop=(kx == K7 - 1))
            st = stg.tile([MC, 512], BF16, tag="stg", name="stk")
            nc.vector.tensor_copy(st[:], acc[:])
            for g in range(K7):
                i0 = max(0, g - 3 - y0)
                i1 = min(KROWS, HK + g - 3 - y0)
                if i1 <= i0:
                    continue
                cnt = (i1 - i0) * HK
                d0 = (y0 + i0 - g + 3) * HK
                nc.sync.dma_start(
                    pk[g * CG:(g + 1) * CG, d0:d0 + cnt],
                    st[g * CG:(g + 1) * CG, i0 * HK:i0 * HK + cnt],
                )

        def k2_build(h):
            k2 = k2p.tile([MC, SK], BF16, tag="k2", name="k2")
            k2_t[h] = k2
            for ch in range(2):
                acc = ps_mm.tile([MC, 512], F32, tag="mm", name="acc2")
                nc.tensor.matmul(acc[:], repl_sb[:],
                                 pk_t[h][:, ch * 512:(ch + 1) * 512],
                                 start=True, stop=True)
                nc.vector.tensor_copy(k2[:, ch * 512:(ch + 1) * 512], acc[:])

        def conv_q_chunk(h, ch):
            q128 = (qA, qB)[h]
            sl = slice(0, 64) if h == 0 else slice(64, 128)
            if ch == 0:
                pq = pqp.tile([MC, SQ], BF16, tag="pq", name="pq")
                nc.vector.memset(pq[:, 0:3 * HQ], 0.0)
                nc.vector.memset(pq[:, (HQ - 3) * HQ:SQ], 0.0)
                pq_t[h] = pq
            pq = pq_t[h]
            acc = ps_mm.tile([MC, 512], F32, tag="mm", name="accq")
            y0 = ch * QROWS
            for kx in range(K7):
                r1 = q128[:].rearrange("p (r c) -> p r c", c=QP)[
                    :, y0:y0 + QROWS, kx:kx + HQ]
                nc.tensor.matmul(acc[:], wcq_sb[:, kx, :], r1,
                                 start=(kx == 0), stop=False)
            for kx in range(K7):
                r2 = qC[:].rearrange("p (r c) -> p r c", c=QP)[
                    sl, y0:y0 + QROWS, kx:kx + HQ]
                nc.tensor.matmul(acc[:], wcq2_sb[sl, kx, :], r2,
                                 start=False, stop=(kx == K7 - 1))
            st = stg.tile([MC, 512], BF16, tag="stg", name="stq")
            nc.vector.tensor_copy(st[:], acc[:])
            for g in range(K7):
                i0 = max(0, g - 3 - y0)
                i1 = min(QROWS, HQ + g - 3 - y0)
                if i1 <= i0:
                    continue
                cnt = (i1 - i0) * HQ
                d0 = (y0 + i0 - g + 3) * HQ
                nc.sync.dma_start(
                    pq[g * CG:(g + 1) * CG, d0:d0 + cnt],
                    st[g * CG:(g + 1) * CG, i0 * HQ:i0 * HQ + cnt],
                )

        def slab(h, s):
            if s == 0:
                rec_sb_t[h] = rp.tile([65, SQ], F32, tag="recsb", name="recsb")
            e_sb = ep.tile([128, SQ], F32R, tag="e", name="esb")
            for half in range(2):
                sc = ps_sc.tile([128, 2048], F32, tag="sc", name="sc")
                for bb in range(4):
                    blk = half * 4 + bb
                    nc.tensor.matmul(
                        sc[:, bb * 512:(bb + 1) * 512],
                        k2_t[h][:, blk * 128:(blk + 1) * 128],
                        pq_t[h][:, s * 512:(s + 1) * 512],
                        start=True, stop=True,
                    )
                nc.scalar.activation(
                    e_sb[:, half * 2048:(half + 1) * 2048], sc[:],
                    AF.Exp, scale=SCALE)
            rec = ps_rec.tile([65, 512], F32, tag="rec", name="rec")
            for t in range(8):
                nc.tensor.matmul(
                    rec[:],
                    v_sb[:, t, h * 65:(h + 1) * 65],
                    e_sb[:, t * 512:(t + 1) * 512],
                    start=(t == 0), stop=(t == 7),
                )
            nc.vector.tensor_copy(
                rec_sb_t[h][:, s * 512:(s + 1) * 512], rec[:])

        def divide(h):
            rec_sb = rec_sb_t[h]
            # ln Z in place, then 1/Z = exp(-ln Z)
            nc.scalar.activation(rec_sb[64:65, :], rec_sb[64:65, :], AF.Ln)
            zinv = zp.tile([65, SQ], F32R, tag="zinv", name="zinv")
            nc.scalar.activation(zinv[64:65, :], rec_sb[64:65, :],
                                 AF.Exp, scale=-1.0)
            for s in range(NSLAB):
                zb = ps_mm.tile([64, 512], F32, tag="mm", name="zb")
                nc.tensor.matmul(zb[:], ones_t[64:65, :],
                                 zinv[64:65, s * 512:(s + 1) * 512],
                                 start=True, stop=True)
                ot = op.tile([64, 512], F32, tag="ot", name="ot")
                nc.vector.tensor_mul(
                    ot[:], rec_sb[0:64, s * 512:(s + 1) * 512], zb[:])
                nc.sync.dma_start(
                    out[h * 64:(h + 1) * 64, s * 512:(s + 1) * 512], ot[:])

        # ---- schedule: head 0 conv, then slabs(h0) interleaved with
        # head 1 conv, then slabs(h1) ----
        for ch in range(KCH):
            conv_k_chunk(0, ch)
        k2_build(0)
        for ch in range(QCH):
            conv_q_chunk(0, ch)
        for s in range(NSLAB):
            slab(0, s)
            if s < KCH:
                conv_k_chunk(1, s)
            if s == KCH:
                k2_build(1)
            conv_q_chunk(1, s)
        divide(0)
        for s in range(NSLAB):
            slab(1, s)
        divide(1)


@functools.lru_cache(maxsize=1)
def _get_program():
    return _build_program()


def _host_inputs(xq, xkv, wq, wk, wv, emb_q, emb_k):
    """Build the 8 per-core input maps."""
    xq = np.ascontiguousarray(xq, dtype=np.float32)
    xkv = np.ascontiguousarray(xkv, dtype=np.float32)

    def conv_w(emb):
        # emb [cg, d, dnk, ky, kx] -> rows (dnk, d), cols (kx, ky, cg)
        arr = np.transpose(np.asarray(emb, np.float32), (2, 1, 4, 3, 0))
        w128 = np.ascontiguousarray(
            arr[0:2].reshape(128, 7, MC).astype(ml_dtypes.bfloat16))
        w64 = arr[2].reshape(64, 7, MC)
        w64d = np.ascontiguousarray(
            np.concatenate([w64, w64], axis=0).astype(ml_dtypes.bfloat16))
        return w128, w64d

    wcq_, wcq2_ = conv_w(emb_q)
    wck_, wck2_ = conv_w(emb_k)
    repl_ = np.ascontiguousarray(
        np.tile(np.eye(CG, dtype=np.float32), (K7, K7)).astype(
            ml_dtypes.bfloat16))

    wq = np.asarray(wq, np.float32)
    wk = np.asarray(wk, np.float32)
    wv = np.asarray(wv, np.float32)

    in_maps = []
    for core in range(8):
        b, p = divmod(core, 2)
        wqT_ = np.zeros((256, 256), np.float32)
        wkT_ = np.zeros((256, 256), np.float32)
        for j in range(4):
            head = 2 * p + j - 1
            if 0 <= head < NH:
                wqT_[:, j * 64:(j + 1) * 64] = wq[head * 64:(head + 1) * 64, :].T
                wkT_[:, j * 64:(j + 1) * 64] = wk[head * 64:(head + 1) * 64, :].T
        wvT_ = np.ascontiguousarray(wv[p * 128:(p + 1) * 128, :].T)
        in_maps.append(dict(
            xq=np.ascontiguousarray(xq[b].reshape(256, SQ)),
            xkv=np.ascontiguousarray(xkv[b].reshape(256, SK)),
            wqT=wqT_, wkT=wkT_, wvT=wvT_,
            wcq=wcq_, wcq2=wcq2_, wck=wck_, wck2=wck2_,
            repl=repl_,
        ))
    return in_maps


def _run(inputs, **kw):
    nc = _get_program()
    in_maps = _host_inputs(**inputs)
    res = run_bass_kernel_spmd(nc, in_maps, core_ids=list(range(8)), **kw)
    outp = np.empty((B, 256, HQ, HQ), np.float32)
    for core in range(8):
        b, p = divmod(core, 2)
        outp[b, p * 128:(p + 1) * 128] = \
            res.results[core]["out"].reshape(128, HQ, HQ)
    return outp, res


def kernel(xq, xkv, wq, wk, wv, emb_q, emb_k):
    outp, _ = _run(dict(xq=xq, xkv=xkv, wq=wq, wk=wk, wv=wv,
                        emb_q=emb_q, emb_k=emb_k))
    return outp


# revision 10
# speedup vs baseline: 1.3803x; 1.0415x over previous
"""Trainium2 Bass kernel for ChannelSqueezeSpatialAttention.

Reference computation (shapes hardcoded):
  xq  [4, 256, 64, 64], xkv [4, 256, 32, 32]
  wq/wk/wv [256, 256], emb_q/emb_k [17, 64, 3, 7, 7]
  q = wq @ xq (1x1 conv), k = wk @ xkv, v = wv @ xkv
  q_c = conv3d(q, emb_q) over (head, y, x) with kernel (3,7,7) -> 17 ch/head
  k_c = conv3d(k, emb_k)
  sim = softmax(q_c^T k_c / 8), rec = sim @ v  -> [4, 256, 64, 64]

Sharding: 8 cores = 4 batches x 2 head-pairs. Each core computes 2 heads of
one batch. The conv mixes adjacent heads (3-wide along head axis), so each
core computes q/k projections for its pair-relative head slots r0..r3 =
heads (2p-1, 2p, 2p+1, 2p+2); out-of-range slots get zero weight columns
host-side (no halo exchange needed).

Conv mapping: shift-and-accumulate matmuls with M = (ky, cg) = 7*17 = 119
packed output rows. The ky-summation is deferred: partial planes are stored
to SBUF with a per-ky y-shift (small SBUF->SBUF DMAs), and the scores
matmul contracts over (ky, cg) with a ky-replicated k_c as the stationary
operand, which completes the convolution for free.

Attention: scores computed transposed S^T[sk, sq] so softmax-exp output E^T
feeds the value matmul directly: rec^T[d|Z, sq] = [v|1]^T E^T. Division by
Z via ACT ln/exp (1/Z = exp(-ln Z), same table set as the softmax exp) +
K=1 broadcast matmul + DVE multiply.

Dtypes: conv/scores chain in bf16 (PE streams 2-byte dtypes at 1 row/cycle
vs 2 for fp32r); projections, E, and the value matmul in float32r.
"""

import functools
import numpy as np
import ml_dtypes

import concourse.bass as bass
import concourse.tile as tile
import concourse.mybir as mybir
from concourse import bacc
from concourse.bass_utils import run_bass_kernel_spmd

F32 = mybir.dt.float32
F32R = mybir.dt.float32r
BF16 = mybir.dt.bfloat16

B = 4
NH = 4
D = 64            # head dim
CG = 17           # squeezed channels
K7 = 7            # spatial kernel
HQ = 64           # q image h=w
HK = 32           # k image h=w
SQ = HQ * HQ      # 4096
SK = HK * HK      # 1024
MC = K7 * CG      # 119 conv output rows (ky, cg)
QP = HQ + 6       # 70: x-padded q row width
KP = HK + 6       # 38: x-padded k row width
SCALE = D ** -0.5

QCH = 8           # q spatial chunks (8 y-rows each)
KCH = 2           # k spatial chunks (16 y-rows each)
QROWS = HQ // QCH  # 8
KROWS = HK // KCH  # 16
NSLAB = SQ // 512  # 8 sq slabs per head

AF = mybir.ActivationFunctionType


def _build_program():
    nc = bacc.Bacc()

    xq = nc.dram_tensor("xq", [256, SQ], BF16, kind="ExternalInput")
    xkv = nc.dram_tensor("xkv", [256, SK], F32R, kind="ExternalInput")
    wqT = nc.dram_tensor("wqT", [256, 256], BF16, kind="ExternalInput")
    wkT = nc.dram_tensor("wkT", [256, 256], F32R, kind="ExternalInput")
    wvT = nc.dram_tensor("wvT", [256, 128], F32R, kind="ExternalInput")
    wcq = nc.dram_tensor("wcq", [128, 7, MC], BF16, kind="ExternalInput")
    wcq2 = nc.dram_tensor("wcq2", [128, 7, MC], BF16, kind="ExternalInput")
    wck = nc.dram_tensor("wck", [128, 7, MC], BF16, kind="ExternalInput")
    wck2 = nc.dram_tensor("wck2", [128, 7, MC], BF16, kind="ExternalInput")
    repl = nc.dram_tensor("repl", [MC, MC], BF16, kind="ExternalInput")
    out = nc.dram_tensor("out", [128, SQ], F32, kind="ExternalOutput")

    with tile.TileContext(nc) as tc:
        _emit(nc, tc, xq, xkv, wqT, wkT, wvT, wcq, wcq2, wck, wck2, repl, out)
    nc.compile()
    return nc


def _emit(nc, tc, xq, xkv, wqT, wkT, wvT, wcq, wcq2, wck, wck2, repl, out):
    import contextlib
    ctx = contextlib.ExitStack()
    with ctx:
        consts = ctx.enter_context(tc.tile_pool(name="consts", bufs=1))
        xqp = ctx.enter_context(tc.tile_pool(name="xqp", bufs=3))
        stg = ctx.enter_context(tc.tile_pool(name="stg", bufs=3))
        pqp = ctx.enter_context(tc.tile_pool(name="pqp", bufs=2))
        pkp = ctx.enter_context(tc.tile_pool(name="pkp", bufs=2))
        k2p = ctx.enter_context(tc.tile_pool(name="k2p", bufs=2))
        ep = ctx.enter_context(tc.tile_pool(name="ep", bufs=2))
        rp = ctx.enter_context(tc.tile_pool(name="rp", bufs=2))
        zp = ctx.enter_context(tc.tile_pool(name="zp", bufs=2))
        op = ctx.enter_context(tc.tile_pool(name="op", bufs=3))
        ps_mm = ctx.enter_context(tc.tile_pool(name="ps_mm", bufs=2, space="PSUM"))
        ps_sc = ctx.enter_context(tc.tile_pool(name="ps_sc", bufs=1, space="PSUM"))
        ps_rec = ctx.enter_context(tc.tile_pool(name="ps_rec", bufs=2, space="PSUM"))

        # ---- constant loads ----
        wqT_sb = consts.tile([128, 2, 256], BF16)
        wkT_sb = consts.tile([128, 2, 256], F32R)
        wvT_sb = consts.tile([128, 2, 128], F32R)
        nc.sync.dma_start(wqT_sb, wqT.rearrange("(t p) m -> p t m", t=2))
        nc.sync.dma_start(wkT_sb, wkT.rearrange("(t p) m -> p t m", t=2))
        nc.sync.dma_start(wvT_sb, wvT.rearrange("(t p) m -> p t m", t=2))
        wcq_sb = consts.tile([128, 7, MC], BF16)
        wcq2_sb = consts.tile([128, 7, MC], BF16)
        wck_sb = consts.tile([128, 7, MC], BF16)
        wck2_sb = consts.tile([128, 7, MC], BF16)
        nc.sync.dma_start(wcq_sb, wcq[:])
        nc.sync.dma_start(wcq2_sb, wcq2[:])
        nc.sync.dma_start(wck_sb, wck[:])
        nc.sync.dma_start(wck2_sb, wck2[:])
        repl_sb = consts.tile([MC, MC], BF16)
        nc.sync.dma_start(repl_sb, repl[:])
        # ones row at partition 64 (aligned with Z row of rec psum)
        ones_t = consts.tile([65, 64], F32R)
        nc.vector.memset(ones_t[64:65, :].bitcast(F32), 1.0)
        xkv_sb = consts.tile([128, 2, SK], F32R)
        nc.sync.dma_start(xkv_sb, xkv.rearrange("(t p) n -> p t n", t=2))

        # q/k plane tensors: [r0,r1], [r1,r2], [r2,r3]; x-padded (64|32)x(70|38)
        qA = consts.tile([128, HQ * QP], BF16)
        qB = consts.tile([128, HQ * QP], BF16)
        qC = consts.tile([128, HQ * QP], BF16)
        kA = consts.tile([128, HK * KP], BF16)
        kB = consts.tile([128, HK * KP], BF16)
        kC = consts.tile([128, HK * KP], BF16)

        # x-border zeros (cols 0..2 and 67..69 of each padded row); B copies
        # inherit them from A/C.
        for t in (qA, qC):
            v = t[:].rearrange("p (r c) -> p r c", c=QP)
            nc.vector.memset(v[:, :, 0:3], 0.0)
            nc.vector.memset(v[:, :, QP - 3:QP], 0.0)
        for t in (kA, kC):
            v = t[:].rearrange("p (r c) -> p r c", c=KP)
            nc.vector.memset(v[:, :, 0:3], 0.0)
            nc.vector.memset(v[:, :, KP - 3:KP], 0.0)

        # v^T projection: out[sk_blk, (h0 d | h1 d)]
        v_sb = consts.tile([128, 8, 130], F32R)
        nc.vector.memset(v_sb[:, :, 64:65].bitcast(F32), 1.0)
        nc.vector.memset(v_sb[:, :, 129:130].bitcast(F32), 1.0)
        for t in range(8):
            acc = ps_mm.tile([128, 512], F32, tag="mm", name="accv")
            for ct in range(2):
                nc.tensor.matmul(
                    acc[:, 0:128],
                    xkv_sb[:, ct, t * 128:(t + 1) * 128],
                    wvT_sb[:, ct, :],
                    start=(ct == 0), stop=(ct == 1),
                )
            nc.vector.tensor_copy(v_sb[:, t, 0:64], acc[:, 0:64])
            nc.vector.tensor_copy(v_sb[:, t, 65:129], acc[:, 64:128])

        # k projection -> kA/kC padded planes (bf16)
        for ch in range(2):
            for mt in range(2):
                acc = ps_mm.tile([128, 512], F32, tag="mm", name="acckp")
                for ct in range(2):
                    nc.tensor.matmul(
                        acc[:],
                        wkT_sb[:, ct, mt * 128:(mt + 1) * 128],
                        xkv_sb[:, ct, ch * 512:(ch + 1) * 512],
                        start=(ct == 0), stop=(ct == 1),
                    )
                rows = 512 // HK  # 16
                y0 = ch * rows
                full = (kA, kC)[mt]
                dstf = full[:].rearrange("p (r c) -> p r c", c=KP)
                accv = acc[:].rearrange("p (r c) -> p r c", c=HK)
                nc.vector.tensor_copy(dstf[:, y0:y0 + rows, 3:3 + HK], accv)

        # q projection -> qA/qC (bf16)
        for ch in range(QCH):
            xt = [None, None]
            for ct in range(2):
                xt[ct] = xqp.tile([128, 512], BF16, tag="xq", name="xqt")
                nc.sync.dma_start(xt[ct], xq[ct * 128:(ct + 1) * 128,
                                              ch * 512:(ch + 1) * 512])
            for mt in range(2):
                acc = ps_mm.tile([128, 512], F32, tag="mm", name="accqp")
                for ct in range(2):
                    nc.tensor.matmul(
                        acc[:],
                        wqT_sb[:, ct, mt * 128:(mt + 1) * 128],
                        xt[ct][:],
                        start=(ct == 0), stop=(ct == 1),
                    )
                rows = 512 // HQ  # 8
                y0 = ch * rows
                full = (qA, qC)[mt]
                dstf = full[:].rearrange("p (r c) -> p r c", c=QP)
                accv = acc[:].rearrange("p (r c) -> p r c", c=HQ)
                nc.vector.tensor_copy(dstf[:, y0:y0 + rows, 3:3 + HQ], accv)

        # B planes (r1, r2) via partition-shifting SBUF->SBUF DMA
        nc.sync.dma_start(qB[0:64, :], qA[64:128, :])
        nc.sync.dma_start(qB[64:128, :], qC[0:64, :])
        nc.sync.dma_start(kB[0:64, :], kA[64:128, :])
        nc.sync.dma_start(kB[64:128, :], kC[0:64, :])

        # ---- per-head stage emitters ----
        pq_t = [None, None]
        pk_t = [None, None]
        k2_t = [None, None]
        rec_sb_t = [None, None]

        def conv_k_chunk(h, ch):
            k128 = (kA, kB)[h]
            sl = slice(0, 64) if h == 0 else slice(64, 128)
            if ch == 0:
                pk = pkp.tile([MC, SK], BF16, tag="pk", name="pk")
                nc.vector.memset(pk[:, 0:3 * HK], 0.0)
                nc.vector.memset(pk[:, (HK - 3) * HK:SK], 0.0)
                pk_t[h] = pk
            pk = pk_t[h]
            acc = ps_mm.tile([MC, 512], F32, tag="mm", name="acck")
            y0 = ch * KROWS
            for kx in range(K7):
                r1 = k128[:].rearrange("p (r c) -> p r c", c=KP)[
                    :, y0:y0 + KROWS, kx:kx + HK]
                nc.tensor.matmul(acc[:], wck_sb[:, kx, :], r1,
                                 start=(kx == 0), stop=False)
            for kx in range(K7):
                r2 = kC[:].rearrange("p (r c) -> p r c", c=KP)[
                    sl, y0:y0 + KROWS, kx:kx + HK]
                nc.tensor.matmul(acc[:], wck2_sb[sl, kx, :], r2,
                                 start=False, stop=(kx == K7 - 1))
            st = stg.tile([MC, 512], BF16, tag="stg", name="stk")
            nc.vector.tensor_copy(st[:], acc[:])
            for g in range(K7):
                i0 = max(0, g - 3 - y0)
                i1 = min(KROWS, HK + g - 3 - y0)
                if i1 <= i0:
                    continue
                cnt = (i1 - i0) * HK
                d0 = (y0 + i0 - g + 3) * HK
                nc.sync.dma_start(
                    pk[g * CG:(g + 1) * CG, d0:d0 + cnt],
                    st[g * CG:(g + 1) * CG, i0 * HK:i0 * HK + cnt],
                )

        def k2_build(h):
            k2 = k2p.tile([MC, SK], BF16, tag="k2", name="k2")
            k2_t[h] = k2
            for ch in range(2):
                acc = ps_mm.tile([MC, 512], F32, tag="mm", name="acc2")
                nc.tensor.matmul(acc[:], repl_sb[:],
                                 pk_t[h][:, ch * 512:(ch + 1) * 512],
                                 start=True, stop=True)
                nc.vector.tensor_copy(k2[:, ch * 512:(ch + 1) * 512], acc[:])

        def conv_q_chunk(h, ch):
            q128 = (qA, qB)[h]
            sl = slice(0, 64) if h == 0 else slice(64, 128)
            if ch == 0:
                pq = pqp.tile([MC, SQ], BF16, tag="pq", name="pq")
                nc.vector.memset(pq[:, 0:3 * HQ], 0.0)
                nc.vector.memset(pq[:, (HQ - 3) * HQ:SQ], 0.0)
                pq_t[h] = pq
            pq = pq_t[h]
            acc = ps_mm.tile([MC, 512], F32, tag="mm", name="accq")
            y0 = ch * QROWS
            for kx in range(K7):
                r1 = q128[:].rearrange("p (r c) -> p r c", c=QP)[
                    :, y0:y0 + QROWS, kx:kx + HQ]
                nc.tensor.matmul(acc[:], wcq_sb[:, kx, :], r1,
                                 start=(kx == 0), stop=False)
            for kx in range(K7):
                r2 = qC[:].rearrange("p (r c) -> p r c", c=QP)[
                    sl, y0:y0 + QROWS, kx:kx + HQ]
                nc.tensor.matmul(acc[:], wcq2_sb[sl, kx, :], r2,
                                 start=False, stop=(kx == K7 - 1))
            st = stg.tile([MC, 512], BF16, tag="stg", name="stq")
            nc.vector.tensor_copy(st[:], acc[:])
            for g in range(K7):
                i0 = max(0, g - 3 - y0)
                i1 = min(QROWS, HQ + g - 3 - y0)
                if i1 <= i0:
                    continue
                cnt = (i1 - i0) * HQ
                d0 = (y0 + i0 - g + 3) * HQ
                nc.sync.dma_start(
                    pq[g * CG:(g + 1) * CG, d0:d0 + cnt],
                    st[g * CG:(g + 1) * CG, i0 * HQ:i0 * HQ + cnt],
                )

        def slab(h, s):
            if s == 0:
                rec_sb_t[h] = rp.tile([65, SQ], F32, tag="recsb", name="recsb")
            e_sb = ep.tile([128, SQ], F32R, tag="e", name="esb")
            for half in range(2):
                sc = ps_sc.tile([128, 2048], F32, tag="sc", name="sc")
                for bb in range(4):
                    blk = half * 4 + bb
                    nc.tensor.matmul(
                        sc[:, bb * 512:(bb + 1) * 512],
                        k2_t[h][:, blk * 128:(blk + 1) * 128],
                        pq_t[h][:, s * 512:(s + 1) * 512],
                        start=True, stop=True,
                    )
                nc.scalar.activation(
                    e_sb[:, half * 2048:(half + 1) * 2048], sc[:],
                    AF.Exp, scale=SCALE)
            rec = ps_rec.tile([65, 512], F32, tag="rec", name="rec")
            for t in range(8):
                nc.tensor.matmul(
                    rec[:],
                    v_sb[:, t, h * 65:(h + 1) * 65],
                    e_sb[:, t * 512:(t + 1) * 512],
                    start=(t == 0), stop=(t == 7),
                )
            nc.vector.tensor_copy(
                rec_sb_t[h][:, s * 512:(s + 1) * 512], rec[:])

        def divide_q(h, qrt):
            rec_sb = rec_sb_t[h]
            c0 = qrt * 2048
            # ln Z in place, then 1/Z = exp(-ln Z)
            nc.scalar.activation(rec_sb[64:65, c0:c0 + 2048],
                                 rec_sb[64:65, c0:c0 + 2048], AF.Ln)
            zinv = zp.tile([65, 2048], F32R, tag="zinv", name="zinv")
            nc.scalar.activation(zinv[64:65, :], rec_sb[64:65, c0:c0 + 2048],
                                 AF.Exp, scale=-1.0)
            for j in range(4):
                s = qrt * 4 + j
                zb = ps_mm.tile([64, 512], F32, tag="mm", name="zb")
                nc.tensor.matmul(zb[:], ones_t[64:65, :],
                                 zinv[64:65, j * 512:(j + 1) * 512],
                                 start=True, stop=True)
                ot = op.tile([64, 512], F32, tag="ot", name="ot")
                nc.vector.tensor_mul(
                    ot[:], rec_sb[0:64, s * 512:(s + 1) * 512], zb[:])
                nc.sync.dma_start(
                    out[h * 64:(h + 1) * 64, s * 512:(s + 1) * 512], ot[:])

        # ---- schedule: conv(h0); slabs(h0) interleaved with conv(h1),
        # then with slabs(h1); divisions in quartets as they complete ----
        for ch in range(KCH):
            conv_k_chunk(0, ch)
        for ch in range(QCH):
            conv_q_chunk(0, ch)
        k2_build(0)
        for s in range(NSLAB):
            slab(0, s)
            if s < KCH:
                conv_k_chunk(1, s)
            if s == KCH:
                k2_build(1)
            conv_q_chunk(1, s)
            if s == 3:
                divide_q(0, 0)
            if s >= 4:
                slab(1, s - 4)
        divide_q(0, 1)
        slab(1, 4)
        divide_q(1, 0)
        for s in range(5, NSLAB):
            slab(1, s)
        divide_q(1, 1)


@functools.lru_cache(maxsize=1)
def _get_program():
    return _build_program()


def _host_inputs(xq, xkv, wq, wk, wv, emb_q, emb_k):
    """Build the 8 per-core input maps."""
    xq = np.ascontiguousarray(xq, dtype=np.float32)
    xkv = np.ascontiguousarray(xkv, dtype=np.float32)

    def conv_w(emb):
        # emb [cg, d, dnk, ky, kx] -> rows (dnk, d), cols (kx, ky, cg)
        arr = np.transpose(np.asarray(emb, np.float32), (2, 1, 4, 3, 0))
        w128 = np.ascontiguousarray(
            arr[0:2].reshape(128, 7, MC).astype(ml_dtypes.bfloat16))
        w64 = arr[2].reshape(64, 7, MC)
        w64d = np.ascontiguousarray(
            np.concatenate([w64, w64], axis=0).astype(ml_dtypes.bfloat16))
        return w128, w64d

    wcq_, wcq2_ = conv_w(emb_q)
    wck_, wck2_ = conv_w(emb_k)
    repl_ = np.ascontiguousarray(
        np.tile(np.eye(CG, dtype=np.float32), (K7, K7)).astype(
            ml_dtypes.bfloat16))

    wq = np.asarray(wq, np.float32)
    wk = np.asarray(wk, np.float32)
    wv = np.asarray(wv, np.float32)

    in_maps = []
    for core in range(8):
        b, p = divmod(core, 2)
        wqT_ = np.zeros((256, 256), np.float32)
        wkT_ = np.zeros((256, 256), np.float32)
        for j in range(4):
            head = 2 * p + j - 1
            if 0 <= head < NH:
                wqT_[:, j * 64:(j + 1) * 64] = wq[head * 64:(head + 1) * 64, :].T
                wkT_[:, j * 64:(j + 1) * 64] = wk[head * 64:(head + 1) * 64, :].T
        wvT_ = np.ascontiguousarray(wv[p * 128:(p + 1) * 128, :].T)
        in_maps.append(dict(
            xq=np.ascontiguousarray(
                xq[b].reshape(256, SQ).astype(ml_dtypes.bfloat16)),
            xkv=np.ascontiguousarray(xkv[b].reshape(256, SK)),
            wqT=np.ascontiguousarray(wqT_.astype(ml_dtypes.bfloat16)),
            wkT=wkT_, wvT=wvT_,
            wcq=wcq_, wcq2=wcq2_, wck=wck_, wck2=wck2_,
            repl=repl_,
        ))
    return in_maps


def _run(inputs, **kw):
    nc = _get_program()
    in_maps = _host_inputs(**inputs)
    res = run_bass_kernel_spmd(nc, in_maps, core_ids=list(range(8)), **kw)
    outp = np.empty((B, 256, HQ, HQ), np.float32)
    for core in range(8):
        b, p = divmod(core, 2)
        outp[b, p * 128:(p + 1) * 128] = \
            res.results[core]["out"].reshape(128, HQ, HQ)
    return outp, res


def kernel(xq, xkv, wq, wk, wv, emb_q, emb_k):
    outp, _ = _run(dict(xq=xq, xkv=xkv, wq=wq, wk=wk, wv=wv,
                        emb_q=emb_q, emb_k=emb_k))
    return outp
